# revision 71
# baseline (speedup 1.0000x reference)
"""LocallyConnected2D Trainium2 kernel (bf16, SWDGE DMA, PE column tiling).

Problem: out[b,o,h,w] = sum_{c,kh,kw} xpad[b,c,h+kh,w+kw] * W[(c,kh,kw), (h,w), o] + bias[o,h,w]
  B=16, C_IN=32, H=W=64, C_OUT=64, KH=KW=3, pad=1  ->  DEPTH=288, S=4096.

Sharding: S split into 8 contiguous blocks of 512 (8 output rows each), one per
core.  Each core sees the full batch; no cross-core reduction.

The layer is weight-streaming bound (302 MB fp32 read once, ~38 MB/core).
Key implementation choices, each driven by a measured bottleneck:
  - weights/activations stream in bf16 (halves DMA; fp32 accumulate in PSUM;
    ~0.4% output error vs the 2e-2 gate).
  - all bulk DMAs issue from gpsimd (SWDGE): HWDGE rings are serviced by a
    single SDMA engine (~27 GB/s); SWDGE spreads descriptors over all 16.
  - DRAM weight layout gives one contiguous run per (partition row, block);
    each 16-location block streams as two ~48-row sub-DMAs from a 16-deep
    tile pool: one dma_start occupies a single SDMA engine, so many small
    in-flight DMAs are what engage all 16 engines (~220 GB/s sustained).
  - every matmul needs its own stationary operand (locally-connected), so
    LDWEIGHTS+MATMUL pairs dominate PE time.  The 128x128 PE array is split
    into three (128,32) column tiles (tile_position = out PSUM partition base
    32*t, t<3 -- the AP encoding only allows 0/32/64).  Locations are
    processed in TRIPLETS, one location per tile, kh interleaved across the
    triplet, so adjacent pairs always target different PE tiles and the
    ~173ns SBUF access latency pipelines instead of serializing.
  - PSUM start_tensor_calc lazily zeroes a whole 2KB bank row on the written
    partitions, so concurrent accumulation groups must differ in bank or
    partition range: each triplet owns one full bank (cols 0:64, partition
    ranges 32t..32t+16), rotating over 6 banks via the tile pool.

Per-core structure (S_SH=512 locations; weight blocks of SBW=16; 171
location-triplets, the last one ragged):
  - xs3[97, 10560] bf16: rows 32*kw+c = input channel c shifted kw left
    (transposed, padded, batch-major); row 96 = ones (bias row).
  - wk[97, blk(8), kh(3), 64, 64] bf16 streamed per block (prefetched one
    block ahead); row 96 of kh=2 carries bias[s,o], fusing the bias add.
  - triplet m, member j (t=j): location s = 3m+j, psum[32t:32t+16, 0:64],
    3 accumulating matmuls (K=96/96/97).
  - per triplet one [80,64] PSUM->stage copy (fp32->bf16), alternating
    DVE / Activation engines; stage (32 triplets) is DMA'd out as
    out[48 rows = t*16+b, m*64+o]; the host un-permutes to (B,C_OUT,H,W).
"""

import numpy as np

# ---------------- problem constants (hardcoded; kernel.py must be self-contained) ---
B = 16
C_IN = 32
H = W = 64
C_OUT = 64
KH = KW = 3
S = H * W                     # 4096
N_CORES = 8
S_SH = S // N_CORES           # 512 output locations per core
ROWS_SH = S_SH // W           # 8 output rows per core
IN_ROWS = ROWS_SH + 2         # 10 padded input rows per core
WPAD = W + 2                  # 66
XS_F = B * IN_ROWS * WPAD     # 10560 free elements of xs
K1 = KW * C_IN                # 96  contraction rows per kh chunk
SBW = 16                      # weight-stream block size (locations per block)
NBLK = S_SH // SBW            # 8 blocks (one output row each)
NT = 3                        # PE column tiles / locations per triplet
TPT = 4                       # triplets per PSUM tile (one bank each)
NTRIP = 172                   # ceil(512/3)=171 triplets, padded to 4|NTRIP
NTILE = NTRIP // TPT          # 43 PSUM tiles
TPG = 32                      # triplets per stage group / output DMA
NGRP = (NTRIP + TPG - 1) // TPG  # 6 stage groups (last ragged)

TRACE = False                 # test.py sets True to get an NTFF profile
LAST_RESULTS = None           # BassKernelResults of the last run (for test.py)

_CACHE = {}


def _build_nc():
    import concourse.mybir as mybir
    from concourse import bacc
    from concourse.tile import TileContext

    fp32 = mybir.dt.float32
    bf16 = mybir.dt.bfloat16
    nc = bacc.Bacc(None)

    xs_d = nc.dram_tensor("xs", [K1 + 1, XS_F], bf16, kind="ExternalInput")
    wk_d = nc.dram_tensor(
        "wk", [K1 + 1, NBLK * KH * SBW * C_OUT], bf16, kind="ExternalInput"
    )
    # rows t*16+b; cols m*64+o  (location s = 3m+t)
    out_d = nc.dram_tensor(
        "out", [NT * B, NGRP * TPG * C_OUT], bf16, kind="ExternalOutput"
    )
    out_dr = None

    with TileContext(nc) as tc:
        with (
            tc.tile_pool(name="xs3", bufs=1) as xs3_pool,
            tc.tile_pool(name="wk", bufs=8) as wk_pool,
            tc.tile_pool(name="stage", bufs=3) as stage_pool,
            tc.tile_pool(name="psum", bufs=2, space="PSUM") as psum_pool,
        ):
            # one dma_start = one SDMA engine (~27 GB/s): split every bulk
            # transfer into row-wise sub-DMAs so many engines run in parallel.
            # xs gates all compute: spread it over the SP/Act HWDGE queues
            # (whose sequencers run parallel to the Q7 SWDGE generator) plus
            # four Q7 subs, so it lands during the Q7's gen ramp
            xs3 = xs3_pool.tile([K1 + 1, XS_F], bf16)
            nc.sync.dma_start(out=xs3[0:24, :], in_=xs_d[0:24, :])
            nc.scalar.dma_start(out=xs3[24:48, :], in_=xs_d[24:48, :])

            # view of xs3 as [p, b, f] where f = h*66 + w
            xs3r = xs3[:].rearrange("p (b f) -> p b f", b=B)
            # dram weights viewed as [p, blk, kh*SBW*C_OUT]
            wk_dr = wk_d[:].rearrange("p (blk f) -> p blk f", blk=NBLK)
            out_dr = out_d[:].rearrange("(t b) f -> t b f", b=B)

            wkts = {}

            def ensure_block(k):
                if k >= NBLK or k in wkts:
                    return
                wkt = wk_pool.tile(
                    [K1 + 1, KH * SBW * C_OUT], bf16, tag="wk", name=f"wkt_{k}"
                )
                # 2 subs per 0.59MB block: fine release quanta keep the
                # stream smooth at ~220 GB/s while bounding Q7 SWDGE
                # generation cost (measured optimum vs 1/3/5-sub splits
                # and 8/32/64-location blocks)
                step = 25 if k == 0 else 49
                for r0 in range(0, K1 + 1, step):
                    r1 = min(r0 + step, K1 + 1)
                    if K1 + 1 - r1 < step // 2:
                        r1 = K1 + 1
                    nc.gpsimd.dma_start(
                        out=wkt[r0:r1, :], in_=wk_dr[r0:r1, k, :]
                    )
                    if r1 == K1 + 1:
                        break
                wkts[k] = wkt[:].rearrange("p (k f) -> p k f", k=KH)

            # compute-gating DMAs first on the Q7 queue, finely split so the
            # critical bytes own many engine slots ahead of the weight flood:
            # the Q7 share of xs (its SP/Act shares stream in parallel on
            # their own queues), then weight blocks 0-1
            for r0 in range(48, 90, 6):
                nc.gpsimd.dma_start(out=xs3[r0:r0 + 6, :], in_=xs_d[r0:r0 + 6, :])
            nc.gpsimd.dma_start(out=xs3[90:97, :], in_=xs_d[90:97, :])
            ensure_block(0)
            ensure_block(1)

            stage = None
            ps = None
            ncopy = 0
            for m in range(NTRIP):
                # padded locations recompute loc 511; host drops s >= 512
                locs = [min(NT * m + j, S_SH - 1) for j in range(NT)]
                blk0 = locs[0] // SBW
                ensure_block(blk0)
                ensure_block(blk0 + 1)   # prefetch next weight block

                k, mk = divmod(m, TPT)   # PSUM tile index, bank within tile
                if mk == 0:
                    ps = psum_pool.tile([128, TPT * 512], fp32)  # 4 banks
                if m % TPG == 0:
                    stage = stage_pool.tile([96, TPG * C_OUT], bf16)

                for kh in range(KH):
                    kk = K1 + 1 if kh == 2 else K1
                    for t, s in enumerate(locs):
                        h, w = divmod(s, W)
                        lhsT = xs3r[0:kk, :, (h + kh) * WPAD + w]
                        rhs = wkts[s // SBW][
                            0:kk, kh, (s % SBW) * C_OUT:(s % SBW + 1) * C_OUT
                        ]
                        nc.tensor.matmul(
                            ps[32 * t:32 * t + B,
                               mk * 512:mk * 512 + C_OUT],
                            lhsT,
                            rhs,
                            start=(kh == 0),
                            stop=(kh == KH - 1),
                            # groups in one bank live on disjoint partition
                            # ranges; the sim's zero-region check is
                            # partition-blind, its data model is not
                            skip_group_check=True,
                        )

                if mk == TPT - 1:
                    # tile complete: one copy per lane, banks gathered by a
                    # free-dim strided AP (no partition striding)
                    kg = k % (TPG // TPT)
                    for t in range(NT):
                        src = ps[32 * t:32 * t + B, :].rearrange(
                            "p (bank f) -> p bank f", bank=TPT
                        )[:, :, 0:C_OUT]
                        dst = stage[32 * t:32 * t + B,
                                    kg * TPT * C_OUT:(kg + 1) * TPT * C_OUT]
                        if ncopy % 2 == 0:
                            nc.vector.tensor_copy(
                                dst.rearrange("p (bank f) -> p bank f", bank=TPT),
                                src,
                            )
                        else:
                            nc.scalar.copy(
                                dst.rearrange("p (bank f) -> p bank f", bank=TPT),
                                src,
                            )
                        ncopy += 1

                if m % TPG == TPG - 1 or m == NTRIP - 1:
                    g = m // TPG
                    mdone = min(TPG, NTRIP - g * TPG)   # triplets in group
                    for t in range(NT):
                        # HWDGE queues (SP/Act) are otherwise idle; keep the
                        # Q7 SWDGE generator free for weight streaming
                        eng = nc.sync if (g * NT + t) % 2 == 0 else nc.scalar
                        eng.dma_start(
                            out=out_dr[t, :, g * TPG * C_OUT:
                                       g * TPG * C_OUT + mdone * C_OUT],
                            in_=stage[32 * t:32 * t + B, 0:mdone * C_OUT],
                        )
    return nc


def _prep_inputs(x, weights, bias):
    """Host-side shard + regather + bf16 cast.  Returns list of 8 in_maps."""
    import ml_dtypes

    bf16 = ml_dtypes.bfloat16
    x = np.ascontiguousarray(x, dtype=np.float32)
    w = np.ascontiguousarray(weights, dtype=np.float32).reshape(
        C_IN, KH, KW, S, C_OUT
    )
    bias_t = np.ascontiguousarray(bias, dtype=np.float32).reshape(C_OUT, S).T  # (S, 64)

    xp = np.zeros((B, C_IN, H + 2, WPAD), dtype=np.float32)
    xp[:, :, 1:H + 1, 1:W + 1] = x
    xs_all = xp.transpose(1, 0, 2, 3)  # (c, b, h, w)

    # weights regathered to (kh, d, s, o) with d = (kw, c), once
    w_bf = np.zeros((KH, K1 + 1, S, C_OUT), dtype=bf16)
    for kh in range(KH):
        w_bf[kh, 0:K1] = (
            w[:, kh, :, :, :].transpose(1, 0, 2, 3).reshape(K1, S, C_OUT)
        )
    w_bf[KH - 1, K1] = bias_t

    in_maps = []
    for i in range(N_CORES):
        r0 = i * ROWS_SH
        xs_c = np.ascontiguousarray(xs_all[:, :, r0:r0 + IN_ROWS, :]).reshape(C_IN, XS_F)
        # xs3: rows 32*kw+c = channel c shifted kw elements left; row 96 = ones
        xs3 = np.zeros((K1 + 1, XS_F), dtype=bf16)
        xs3[0:C_IN] = xs_c
        xs3[C_IN:2 * C_IN, 0:XS_F - 1] = xs_c[:, 1:]
        xs3[2 * C_IN:3 * C_IN, 0:XS_F - 2] = xs_c[:, 2:]
        xs3[K1] = np.float32(1.0)
        s0 = i * S_SH
        # wk4[d, blk, kh, s_in_blk, o]
        wk4 = w_bf[:, :, s0:s0 + S_SH, :].reshape(
            KH, K1 + 1, NBLK, SBW, C_OUT
        ).transpose(1, 2, 0, 3, 4)
        in_maps.append({
            "xs": xs3,
            "wk": np.ascontiguousarray(wk4).reshape(K1 + 1, NBLK * KH * SBW * C_OUT),
        })
    return in_maps


def _decode_core(raw):
    """[48, NGRP*TPG*C_OUT] triplet layout -> (B, C_OUT, ROWS_SH, W)."""
    oc = np.asarray(raw).astype(np.float32)
    oc = oc.reshape(NT, B, NGRP * TPG, C_OUT)      # (t, b, m, o)
    oc = oc.transpose(1, 3, 2, 0)                  # (b, o, m, t)
    oc = oc.reshape(B, C_OUT, NGRP * TPG * NT)[:, :, :S_SH]
    return np.ascontiguousarray(oc).reshape(B, C_OUT, ROWS_SH, W)


def _ensure_ntff_hook():
    """Some agent images lack antenv.axon_hooks; synthesize it (plus the
    ctypes NTFF hook from trn_agent_boot) so trace=True can profile.
    No-op when the real module exists or the boot package is absent."""
    import sys
    import types

    try:
        from antenv.axon_hooks import get_axon_ntff_profile_hook  # noqa: F401
        return
    except ImportError:
        pass
    try:
        import antenv
    except ImportError:
        return
    mod = types.ModuleType("antenv.axon_hooks")
    state = {"hook": None}
    mod.set_axon_ntff_profile_hook = lambda h: state.__setitem__("hook", h)
    mod.get_axon_ntff_profile_hook = lambda: state["hook"]
    sys.modules["antenv.axon_hooks"] = mod
    antenv.axon_hooks = mod
    try:
        from trn_agent_boot.trn_boot import _ntff_profile_via_ctypes

        hook = _ntff_profile_via_ctypes("/opt/axon/libaxon_pjrt.so")
        if hook is not None:
            mod.set_axon_ntff_profile_hook(hook)
    except Exception:
        pass


def kernel(x, weights, bias):
    global LAST_RESULTS
    from concourse.bass_utils import run_bass_kernel_spmd

    if TRACE:
        _ensure_ntff_hook()

    if "nc" not in _CACHE:
        nc = _build_nc()
        if not nc.is_finalized():
            nc.finalize()
        _CACHE["nc"] = nc
    nc = _CACHE["nc"]

    in_maps = _prep_inputs(x, weights, bias)
    res = run_bass_kernel_spmd(
        nc, in_maps, core_ids=list(range(N_CORES)), trace=TRACE
    )
    LAST_RESULTS = res

    out = np.empty((B, C_OUT, H, W), dtype=np.float32)
    for i in range(N_CORES):
        oc = _decode_core(res.results[i]["out"])
        out[:, :, i * ROWS_SH:(i + 1) * ROWS_SH, :] = oc
    return out


# revision 72
# speedup vs baseline: 1.0030x; 1.0030x over previous
"""LocallyConnected2D Trainium2 kernel (bf16, SWDGE DMA, PE column tiling).

Problem: out[b,o,h,w] = sum_{c,kh,kw} xpad[b,c,h+kh,w+kw] * W[(c,kh,kw), (h,w), o] + bias[o,h,w]
  B=16, C_IN=32, H=W=64, C_OUT=64, KH=KW=3, pad=1  ->  DEPTH=288, S=4096.

Sharding: S split into 8 contiguous blocks of 512 (8 output rows each), one per
core.  Each core sees the full batch; no cross-core reduction.

The layer is weight-streaming bound (302 MB fp32 read once, ~38 MB/core).
Key implementation choices, each driven by a measured bottleneck:
  - weights/activations stream in bf16 (halves DMA; fp32 accumulate in PSUM;
    ~0.4% output error vs the 2e-2 gate).
  - all bulk DMAs issue from gpsimd (SWDGE): HWDGE rings are serviced by a
    single SDMA engine (~27 GB/s); SWDGE spreads descriptors over all 16.
  - DRAM weight layout gives one contiguous run per (partition row, block);
    each 16-location block streams as two ~48-row sub-DMAs from a 16-deep
    tile pool: one dma_start occupies a single SDMA engine, so many small
    in-flight DMAs are what engage all 16 engines (~220 GB/s sustained).
  - every matmul needs its own stationary operand (locally-connected), so
    LDWEIGHTS+MATMUL pairs dominate PE time.  The 128x128 PE array is split
    into three (128,32) column tiles (tile_position = out PSUM partition base
    32*t, t<3 -- the AP encoding only allows 0/32/64).  Locations are
    processed in TRIPLETS, one location per tile, kh interleaved across the
    triplet, so adjacent pairs always target different PE tiles and the
    ~173ns SBUF access latency pipelines instead of serializing.
  - PSUM start_tensor_calc lazily zeroes a whole 2KB bank row on the written
    partitions, so concurrent accumulation groups must differ in bank or
    partition range: each triplet owns one full bank (cols 0:64, partition
    ranges 32t..32t+16), rotating over 6 banks via the tile pool.

Per-core structure (S_SH=512 locations; weight blocks of SBW=16; 171
location-triplets, the last one ragged):
  - xs3[97, 10560] bf16: rows 32*kw+c = input channel c shifted kw left
    (transposed, padded, batch-major); row 96 = ones (bias row).
  - wk[97, blk(8), kh(3), 64, 64] bf16 streamed per block (prefetched one
    block ahead); row 96 of kh=2 carries bias[s,o], fusing the bias add.
  - triplet m, member j (t=j): location s = 3m+j, psum[32t:32t+16, 0:64],
    3 accumulating matmuls (K=96/96/97).
  - per triplet one [80,64] PSUM->stage copy (fp32->bf16), alternating
    DVE / Activation engines; stage (32 triplets) is DMA'd out as
    out[48 rows = t*16+b, m*64+o]; the host un-permutes to (B,C_OUT,H,W).
"""

import numpy as np

# ---------------- problem constants (hardcoded; kernel.py must be self-contained) ---
B = 16
C_IN = 32
H = W = 64
C_OUT = 64
KH = KW = 3
S = H * W                     # 4096
N_CORES = 8
S_SH = S // N_CORES           # 512 output locations per core
ROWS_SH = S_SH // W           # 8 output rows per core
IN_ROWS = ROWS_SH + 2         # 10 padded input rows per core
WPAD = W + 2                  # 66
XS_F = B * IN_ROWS * WPAD     # 10560 free elements of xs
K1 = KW * C_IN                # 96  contraction rows per kh chunk
SBW = 16                      # weight-stream block size (locations per block)
NBLK = S_SH // SBW            # 8 blocks (one output row each)
NT = 3                        # PE column tiles / locations per triplet
TPT = 4                       # triplets per PSUM tile (one bank each)
NTRIP = 172                   # ceil(512/3)=171 triplets, padded to 4|NTRIP
NTILE = NTRIP // TPT          # 43 PSUM tiles
TPG = 32                      # triplets per stage group / output DMA
NGRP = (NTRIP + TPG - 1) // TPG  # 6 stage groups (last ragged)

TRACE = False                 # test.py sets True to get an NTFF profile
LAST_RESULTS = None           # BassKernelResults of the last run (for test.py)

_CACHE = {}


def _build_nc():
    import concourse.mybir as mybir
    from concourse import bacc
    from concourse.tile import TileContext

    fp32 = mybir.dt.float32
    bf16 = mybir.dt.bfloat16
    nc = bacc.Bacc(None)

    xs_d = nc.dram_tensor("xs", [K1 + 1, XS_F], bf16, kind="ExternalInput")
    wk_d = nc.dram_tensor(
        "wk", [K1 + 1, NBLK * KH * SBW * C_OUT], bf16, kind="ExternalInput"
    )
    # rows t*16+b; cols m*64+o  (location s = 3m+t)
    out_d = nc.dram_tensor(
        "out", [NT * B, NGRP * TPG * C_OUT], bf16, kind="ExternalOutput"
    )
    out_dr = None

    with TileContext(nc) as tc:
        with (
            tc.tile_pool(name="xs3", bufs=1) as xs3_pool,
            tc.tile_pool(name="wk", bufs=16) as wk_pool,
            tc.tile_pool(name="stage", bufs=3) as stage_pool,
            tc.tile_pool(name="psum", bufs=2, space="PSUM") as psum_pool,
        ):
            # one dma_start = one SDMA engine (~27 GB/s): split every bulk
            # transfer into row-wise sub-DMAs so many engines run in parallel.
            # xs gates all compute: 24-row chunks ride the SP/Act HWDGE
            # queues (sequencers parallel to the Q7 SWDGE generator); the
            # rest goes to the Q7 queue head below
            xs3 = xs3_pool.tile([K1 + 1, XS_F], bf16)
            nc.sync.dma_start(out=xs3[0:24, :], in_=xs_d[0:24, :])
            nc.scalar.dma_start(out=xs3[24:48, :], in_=xs_d[24:48, :])

            # view of xs3 as [p, b, f] where f = h*66 + w
            xs3r = xs3[:].rearrange("p (b f) -> p b f", b=B)
            # dram weights viewed as [p, blk, kh*SBW*C_OUT]
            wk_dr = wk_d[:].rearrange("p (blk f) -> p blk f", blk=NBLK)
            out_dr = out_d[:].rearrange("(t b) f -> t b f", b=B)

            wkts = {}

            def ensure_block(k):
                if k >= NBLK or k in wkts:
                    return
                wkt = wk_pool.tile(
                    [K1 + 1, KH * SBW * C_OUT], bf16, tag="wk", name=f"wkt_{k}"
                )
                # 2 subs per 0.59MB block: fine release quanta keep the
                # stream smooth at ~220 GB/s while bounding Q7 SWDGE
                # generation cost (measured optimum vs 1/3/5-sub splits
                # and 8/32/64-location blocks)
                step = 25 if k == 0 else 49
                for r0 in range(0, K1 + 1, step):
                    r1 = min(r0 + step, K1 + 1)
                    if K1 + 1 - r1 < step // 2:
                        r1 = K1 + 1
                    nc.gpsimd.dma_start(
                        out=wkt[r0:r1, :], in_=wk_dr[r0:r1, k, :]
                    )
                    if r1 == K1 + 1:
                        break
                wkts[k] = wkt[:].rearrange("p (k f) -> p k f", k=KH)

            # compute-gating DMAs first on the Q7 queue, finely split so the
            # critical bytes own many engine slots ahead of the weight flood:
            # the Q7 share of xs (its SP/Act shares stream in parallel on
            # their own queues), then weight blocks 0-1
            for r0 in range(48, 90, 6):
                nc.gpsimd.dma_start(out=xs3[r0:r0 + 6, :], in_=xs_d[r0:r0 + 6, :])
            nc.gpsimd.dma_start(out=xs3[90:97, :], in_=xs_d[90:97, :])
            ensure_block(0)
            ensure_block(1)

            stage = None
            ps = None
            ncopy = 0
            for m in range(NTRIP):
                # padded locations recompute loc 511; host drops s >= 512
                locs = [min(NT * m + j, S_SH - 1) for j in range(NT)]
                blk0 = locs[0] // SBW
                ensure_block(blk0)
                ensure_block(blk0 + 1)   # prefetch next weight block

                k, mk = divmod(m, TPT)   # PSUM tile index, bank within tile
                if mk == 0:
                    ps = psum_pool.tile([128, TPT * 512], fp32)  # 4 banks
                if m % TPG == 0:
                    stage = stage_pool.tile([96, TPG * C_OUT], bf16)

                for kh in range(KH):
                    kk = K1 + 1 if kh == 2 else K1
                    for t, s in enumerate(locs):
                        h, w = divmod(s, W)
                        lhsT = xs3r[0:kk, :, (h + kh) * WPAD + w]
                        rhs = wkts[s // SBW][
                            0:kk, kh, (s % SBW) * C_OUT:(s % SBW + 1) * C_OUT
                        ]
                        nc.tensor.matmul(
                            ps[32 * t:32 * t + B,
                               mk * 512:mk * 512 + C_OUT],
                            lhsT,
                            rhs,
                            start=(kh == 0),
                            stop=(kh == KH - 1),
                            # groups in one bank live on disjoint partition
                            # ranges; the sim's zero-region check is
                            # partition-blind, its data model is not
                            skip_group_check=True,
                        )

                if mk == TPT - 1:
                    # tile complete: one copy per lane, banks gathered by a
                    # free-dim strided AP (no partition striding)
                    kg = k % (TPG // TPT)
                    for t in range(NT):
                        src = ps[32 * t:32 * t + B, :].rearrange(
                            "p (bank f) -> p bank f", bank=TPT
                        )[:, :, 0:C_OUT]
                        dst = stage[32 * t:32 * t + B,
                                    kg * TPT * C_OUT:(kg + 1) * TPT * C_OUT]
                        if ncopy % 2 == 0:
                            nc.vector.tensor_copy(
                                dst.rearrange("p (bank f) -> p bank f", bank=TPT),
                                src,
                            )
                        else:
                            nc.scalar.copy(
                                dst.rearrange("p (bank f) -> p bank f", bank=TPT),
                                src,
                            )
                        ncopy += 1

                if m % TPG == TPG - 1 or m == NTRIP - 1:
                    g = m // TPG
                    mdone = min(TPG, NTRIP - g * TPG)   # triplets in group
                    for t in range(NT):
                        # HWDGE queues (SP/Act) are otherwise idle; keep the
                        # Q7 SWDGE generator free for weight streaming
                        eng = nc.sync if (g * NT + t) % 2 == 0 else nc.scalar
                        eng.dma_start(
                            out=out_dr[t, :, g * TPG * C_OUT:
                                       g * TPG * C_OUT + mdone * C_OUT],
                            in_=stage[32 * t:32 * t + B, 0:mdone * C_OUT],
                        )
    return nc


def _prep_inputs(x, weights, bias):
    """Host-side shard + regather + bf16 cast.  Returns list of 8 in_maps."""
    import ml_dtypes

    bf16 = ml_dtypes.bfloat16
    x = np.ascontiguousarray(x, dtype=np.float32)
    w = np.ascontiguousarray(weights, dtype=np.float32).reshape(
        C_IN, KH, KW, S, C_OUT
    )
    bias_t = np.ascontiguousarray(bias, dtype=np.float32).reshape(C_OUT, S).T  # (S, 64)

    xp = np.zeros((B, C_IN, H + 2, WPAD), dtype=np.float32)
    xp[:, :, 1:H + 1, 1:W + 1] = x
    xs_all = xp.transpose(1, 0, 2, 3)  # (c, b, h, w)

    # weights regathered to (kh, d, s, o) with d = (kw, c), once
    w_bf = np.zeros((KH, K1 + 1, S, C_OUT), dtype=bf16)
    for kh in range(KH):
        w_bf[kh, 0:K1] = (
            w[:, kh, :, :, :].transpose(1, 0, 2, 3).reshape(K1, S, C_OUT)
        )
    w_bf[KH - 1, K1] = bias_t

    in_maps = []
    for i in range(N_CORES):
        r0 = i * ROWS_SH
        xs_c = np.ascontiguousarray(xs_all[:, :, r0:r0 + IN_ROWS, :]).reshape(C_IN, XS_F)
        # xs3: rows 32*kw+c = channel c shifted kw elements left; row 96 = ones
        xs3 = np.zeros((K1 + 1, XS_F), dtype=bf16)
        xs3[0:C_IN] = xs_c
        xs3[C_IN:2 * C_IN, 0:XS_F - 1] = xs_c[:, 1:]
        xs3[2 * C_IN:3 * C_IN, 0:XS_F - 2] = xs_c[:, 2:]
        xs3[K1] = np.float32(1.0)
        s0 = i * S_SH
        # wk4[d, blk, kh, s_in_blk, o]
        wk4 = w_bf[:, :, s0:s0 + S_SH, :].reshape(
            KH, K1 + 1, NBLK, SBW, C_OUT
        ).transpose(1, 2, 0, 3, 4)
        in_maps.append({
            "xs": xs3,
            "wk": np.ascontiguousarray(wk4).reshape(K1 + 1, NBLK * KH * SBW * C_OUT),
        })
    return in_maps


def _decode_core(raw):
    """[48, NGRP*TPG*C_OUT] triplet layout -> (B, C_OUT, ROWS_SH, W)."""
    oc = np.asarray(raw).astype(np.float32)
    oc = oc.reshape(NT, B, NGRP * TPG, C_OUT)      # (t, b, m, o)
    oc = oc.transpose(1, 3, 2, 0)                  # (b, o, m, t)
    oc = oc.reshape(B, C_OUT, NGRP * TPG * NT)[:, :, :S_SH]
    return np.ascontiguousarray(oc).reshape(B, C_OUT, ROWS_SH, W)


def _ensure_ntff_hook():
    """Some agent images lack antenv.axon_hooks; synthesize it (plus the
    ctypes NTFF hook from trn_agent_boot) so trace=True can profile.
    No-op when the real module exists or the boot package is absent."""
    import sys
    import types

    try:
        from antenv.axon_hooks import get_axon_ntff_profile_hook  # noqa: F401
        return
    except ImportError:
        pass
    try:
        import antenv
    except ImportError:
        return
    mod = types.ModuleType("antenv.axon_hooks")
    state = {"hook": None}
    mod.set_axon_ntff_profile_hook = lambda h: state.__setitem__("hook", h)
    mod.get_axon_ntff_profile_hook = lambda: state["hook"]
    sys.modules["antenv.axon_hooks"] = mod
    antenv.axon_hooks = mod
    try:
        from trn_agent_boot.trn_boot import _ntff_profile_via_ctypes

        hook = _ntff_profile_via_ctypes("/opt/axon/libaxon_pjrt.so")
        if hook is not None:
            mod.set_axon_ntff_profile_hook(hook)
    except Exception:
        pass


def kernel(x, weights, bias):
    global LAST_RESULTS
    from concourse.bass_utils import run_bass_kernel_spmd

    if TRACE:
        _ensure_ntff_hook()

    if "nc" not in _CACHE:
        nc = _build_nc()
        if not nc.is_finalized():
            nc.finalize()
        _CACHE["nc"] = nc
    nc = _CACHE["nc"]

    in_maps = _prep_inputs(x, weights, bias)
    res = run_bass_kernel_spmd(
        nc, in_maps, core_ids=list(range(N_CORES)), trace=TRACE
    )
    LAST_RESULTS = res

    out = np.empty((B, C_OUT, H, W), dtype=np.float32)
    for i in range(N_CORES):
        oc = _decode_core(res.results[i]["out"])
        out[:, :, i * ROWS_SH:(i + 1) * ROWS_SH, :] = oc
    return out
